# revision 6
# baseline (speedup 1.0000x reference)
"""Sparse avg-pool (segment mean) for Trainium2, 8 NeuronCores.

Strategy: range-shard coarse ids across cores (core k owns ids
[k*31360, (k+1)*31360)), so no collective is needed.  Host-side prep per
core: prescale each token's features by 1/count(coarse id) (counts fall out
of routing), sort tokens by id, and pack them densely into slots of 128.
Every K=2 consecutive slots (256 tokens) form a group with a data-dependent
base id; the device builds a [128 token, W=32 seg] one-hot from
group-relative ids per slot and the PE accumulates onehot^T @ feats into a
per-group PSUM tile.  Because the sum is prescaled, PSUM holds the mean
directly: ACT copies it to SBUF as bf16 and it DMAs straight out.  Adjacent
groups may overlap in id range; the host adds group outputs into the final
array (partial sums, exact).  W=32 deliberately undercuts the max group
span (40 for uniform data): the ~3% of tokens past the window boundary are
blanked on the device (id -1 never matches) and added on the host during
assembly, like halo cells — trimming output rows shipped from 40 to 32 per
group.

Performance notes (TRN2): the fused is_equal hits the DVE 2x_1p fast mode
(all operands 2-byte, unit last-axis stride) via the [p, group, seg, slot]
one-hot layout and a [128, W, K] materialized iota; LDWEIGHTS+MATMUL pairs
issue every ~40-55 ns when the lhsT free-axis stride is small; input streams
chunk-major from DRAM on the sync HWDGE ring while outputs ride the gpsimd
SWDGE ring (last chunks on sync to cut the tail).  ~126 us vs the 364 us
one-hot-on-DVE baseline; the wall is the 36.7 MB blended bf16 stream at the
~332 GB/s practical DMA ceiling plus ~9 us NEFF startup.
"""
import os
import sys
from dataclasses import dataclass

sys.path.insert(0, "/opt/trn_rl_repo")

import numpy as np

NCORES = 8
C = 64
K = 2          # slots per group
SPG = 128 * K  # tokens per group


@dataclass(frozen=True)
class Cfg:
    n_coarse_pad: int = 250_880
    s_slots: int = 1964   # slots per core (982 groups of 2)
    w: int = 32           # one-hot window width (boundary tokens spill to host)
    spc0: int = 32        # starter chunk size (slots)
    spc: int = 64         # main chunk size (slots)
    drain: int = 16       # groups per PSUM tile / ACT copy
    dma_grp: int = 32     # groups per output DMA

    @property
    def rng(self):
        return self.n_coarse_pad // NCORES

    @property
    def n_grp(self):
        return self.s_slots // K

    @property
    def s_tot(self):
        return self.s_slots * 128


CFG = Cfg()
_nc_cache = {}
LAST_RESULT = None


def chunk_plan(cfg: Cfg):
    """Chunk sizes in slots: two small starter chunks to fill the pipeline
    fast, then large transfers (+ ragged even tail) to amortize DMA setup."""
    rem = cfg.s_slots - 2 * cfg.spc0
    assert rem > 0 and rem % 2 == 0
    plan = [cfg.spc0, cfg.spc0] + [cfg.spc] * (rem // cfg.spc)
    tail = rem % cfg.spc
    if tail:
        plan.append(tail)
    return plan


def build_nc(cfg: Cfg):
    from concourse import bacc, mybir, tile

    bf16 = mybir.dt.bfloat16
    f32 = mybir.dt.float32
    nc = bacc.Bacc("TRN2", target_bir_lowering=False)
    plan = chunk_plan(cfg)
    mspc0 = max(plan)
    feats_ext = nc.declare_dram_parameter(
        "feats", [len(plan), 128, mspc0 * C], bf16, isOutput=False
    )
    ids_ext = nc.declare_dram_parameter(
        "ids", [128, cfg.s_slots], bf16, isOutput=False
    )
    iota_ext = nc.declare_dram_parameter(
        "iota", [128, cfg.w, K], bf16, isOutput=False
    )
    out_ext = nc.declare_dram_parameter(
        "out", [cfg.w, cfg.n_grp, C], bf16, isOutput=True
    )

    mspc = max(plan)
    mcg = mspc // K

    with tile.TileContext(nc) as tc:
        with (
            tc.tile_pool(name="stage", bufs=8) as stagep,
            tc.tile_pool(name="oh", bufs=4) as ohp,
            tc.tile_pool(name="psum", bufs=3, space="PSUM") as psump,
            tc.tile_pool(name="fin", bufs=6) as finp,
            tc.tile_pool(name="cst", bufs=1) as cstp,
        ):
            iota_t = cstp.tile([128, cfg.w, K], bf16)
            nc.sync.dma_start(out=iota_t[:], in_=iota_ext[:])
            ids_t = cstp.tile([128, cfg.s_slots], bf16)
            nc.sync.dma_start(out=ids_t[:], in_=ids_ext[:])

            col = 0
            g = 0
            deferred = []
            for ci, spc in enumerate(plan):
                cg = spc // K
                s0 = col
                blk = stagep.tile([128, mspc, C], bf16, tag="src")
                nc.sync.dma_start(
                    out=blk[:, :spc, :],
                    in_=feats_ext[ci, :, : spc * C].rearrange(
                        "p (t c) -> p t c", c=C
                    ),
                )
                feats_v = blk[:, :spc, :]
                ids_v = ids_t[:, s0 : s0 + spc]
                oh = ohp.tile([128, mcg, cfg.w, K], bf16, tag="oh")
                nc.vector.tensor_tensor(
                    out=oh[:, :cg],
                    in0=ids_v.rearrange("p (g s) -> p g s", g=cg)
                    .unsqueeze(2)
                    .to_broadcast([128, cg, cfg.w, K]),
                    in1=iota_t[:]
                    .unsqueeze(1)
                    .to_broadcast([128, cg, cfg.w, K]),
                    op=mybir.AluOpType.is_equal,
                )
                dma_grp = cfg.drain if ci == len(plan) - 1 else cfg.dma_grp
                for h0 in range(0, cg, dma_grp):
                    hn = min(dma_grp, cg - h0)
                    ot = finp.tile([cfg.w, cfg.dma_grp, C], bf16, tag="ot")
                    for d0 in range(h0, h0 + hn, cfg.drain):
                        dn = min(cfg.drain, h0 + hn - d0)
                        ps = psump.tile([cfg.w, cfg.drain, C], f32, tag="ps")
                        for gi in range(dn):
                            for s in range(K):
                                t = (d0 + gi) * K + s
                                nc.tensor.matmul(
                                    out=ps[:, gi, :],
                                    lhsT=oh[:, d0 + gi, :, s],
                                    rhs=feats_v[:, t, :],
                                    start=(s == 0),
                                    stop=(s == K - 1),
                                )
                        nc.scalar.activation(
                            ot[:, d0 - h0 : d0 - h0 + dn, :],
                            ps[:, :dn, :],
                            mybir.ActivationFunctionType.Copy,
                        )
                    if ci >= len(plan) - 3:
                        # defer: the sync ring is in-order, so a compute-
                        # dependent output here would block the remaining
                        # input chunks queued behind it
                        deferred.append((g + h0, hn, ot))
                    else:
                        nc.gpsimd.dma_start(
                            out=out_ext[:, g + h0 : g + h0 + hn, :],
                            in_=ot[:, :hn, :],
                        )
                col += spc
                g += cg
            for g0, hn, ot in deferred:
                nc.sync.dma_start(
                    out=out_ext[:, g0 : g0 + hn, :],
                    in_=ot[:, :hn, :],
                )
    nc.compile()
    return nc


def shard_inputs(feats, ids, cfg: Cfg):
    """Host: scale by 1/count, route to owner cores, sort by local id,
    pack densely, compute per-group base ids and group-relative ids."""
    import ml_dtypes

    ids = np.asarray(ids, dtype=np.int64).ravel()
    feats = np.asarray(feats, dtype=np.float32)
    cnt = np.bincount(ids, minlength=cfg.n_coarse_pad).astype(np.float32)
    scale = 1.0 / np.maximum(cnt, 1.0)
    feats = feats * scale[ids][:, None]

    owner = ids // cfg.rng
    local = (ids - owner * cfg.rng).astype(np.int64)
    order = np.argsort(owner, kind="stable")
    counts = np.bincount(owner, minlength=NCORES)
    offs = np.zeros(NCORES + 1, np.int64)
    np.cumsum(counts, out=offs[1:])
    feats_sorted = feats[order]
    local_sorted = local[order]

    iota = np.broadcast_to(
        np.arange(cfg.w, dtype=np.float32)[None, :, None], (128, cfg.w, K)
    ).astype(ml_dtypes.bfloat16)
    iota = np.ascontiguousarray(iota)

    in_maps = []
    bases_all = []
    spill_idx = []
    spill_val = []
    for k in range(NCORES):
        fk = feats_sorted[offs[k] : offs[k + 1]]
        lk = local_sorted[offs[k] : offs[k + 1]]
        n_k = lk.shape[0]
        assert n_k <= cfg.s_tot, f"core {k}: {n_k} tokens > {cfg.s_tot}"
        sorder = np.argsort(lk, kind="stable")
        ls = lk[sorder]
        fs = fk[sorder]
        bases = np.zeros(cfg.n_grp, np.int64)
        ngrp_used = -(-n_k // SPG)
        bases[:ngrp_used] = ls[np.arange(ngrp_used) * SPG]
        rel = ls - bases[np.arange(n_k) // SPG]
        # tokens whose group-relative id falls outside the one-hot window are
        # rare (never, for uniform-random ids); blank them in place (the
        # device ignores id -1) and accumulate them on the host instead
        over = rel >= cfg.w
        if over.any():
            spill_idx.append(k * cfg.rng + ls[over])
            spill_val.append(fs[over].copy())
            fs = fs.copy()
            rel = rel.copy()
            fs[over] = 0.0
            rel[over] = -1
        fa = np.zeros((cfg.s_tot, C), np.float32)
        ia = np.full((cfg.s_tot,), -1.0, np.float32)
        fa[:n_k] = fs
        ia[:n_k] = rel.astype(np.float32)
        fa = fa.reshape(cfg.s_slots, 128, C).transpose(1, 0, 2)  # [128, S, C]
        plan = chunk_plan(cfg)
        mspc = max(plan)
        feats_arr = np.zeros((len(plan), 128, mspc * C), ml_dtypes.bfloat16)
        s0 = 0
        for ci, spc in enumerate(plan):
            feats_arr[ci, :, : spc * C] = fa[:, s0 : s0 + spc, :].reshape(
                128, spc * C
            )
            s0 += spc
        ids_arr = np.ascontiguousarray(
            ia.reshape(cfg.s_slots, 128).T
        ).astype(ml_dtypes.bfloat16)
        in_maps.append({"feats": feats_arr, "ids": ids_arr, "iota": iota})
        bases_all.append(bases)
    if spill_idx:
        spill = (np.concatenate(spill_idx), np.concatenate(spill_val))
    else:
        spill = None
    return in_maps, bases_all, spill


def assemble_output(results, bases_all, spill, n_coarse, cfg: Cfg):
    out = np.empty((NCORES * cfg.rng, C), np.float32)
    for k in range(NCORES):
        dev = np.asarray(results[k]["out"], dtype=np.float32)  # [W, n_grp, C]
        acc = np.zeros((cfg.rng + cfg.w, C), np.float32)
        bases = bases_all[k]
        for g in range(cfg.n_grp):
            b = bases[g]
            acc[b : b + cfg.w] += dev[:, g, :]
        out[k * cfg.rng : (k + 1) * cfg.rng] = acc[: cfg.rng]
    if spill is not None:
        np.add.at(out, spill[0], spill[1])
    return out[:n_coarse]


def _install_axon_hooks_shim():
    """Provide antenv.axon_hooks + the ctypes NTFF hook if the image lacks it."""
    import contextlib
    import ctypes
    import types

    try:
        from antenv.axon_hooks import get_axon_ntff_profile_hook  # noqa: F401

        return
    except ImportError:
        pass
    import antenv

    mod = types.ModuleType("antenv.axon_hooks")
    state = {"h": None}
    mod.set_axon_ntff_profile_hook = lambda h: state.__setitem__("h", h)
    mod.get_axon_ntff_profile_hook = lambda: state["h"]
    antenv.axon_hooks = mod
    sys.modules["antenv.axon_hooks"] = mod

    so_path = "/opt/axon/libaxon_pjrt.so"
    if not os.path.exists(so_path):
        return
    lib = ctypes.CDLL(so_path)
    if not hasattr(lib, "axon_start_nrt_profile"):
        return
    lib.axon_start_nrt_profile.argtypes = [
        ctypes.POINTER(ctypes.c_int64),
        ctypes.c_size_t,
    ]
    lib.axon_start_nrt_profile.restype = ctypes.c_int64
    lib.axon_stop_nrt_profile.argtypes = [ctypes.c_char_p]
    lib.axon_stop_nrt_profile.restype = ctypes.c_int64

    @contextlib.contextmanager
    def _hook(output_dir, device_ids):
        import jax

        jax.devices()
        if device_ids:
            ids = (ctypes.c_int64 * len(device_ids))(*device_ids)
            rc = lib.axon_start_nrt_profile(ids, len(device_ids))
        else:
            rc = lib.axon_start_nrt_profile(None, 0)
        if rc != 0:
            raise RuntimeError(f"axon_start_nrt_profile rc={rc}")
        try:
            yield
        finally:
            n = lib.axon_stop_nrt_profile(str(output_dir).encode())
            print(f"profile: {n} file(s) written to {output_dir}", file=sys.stderr)

    state["h"] = _hook


def kernel(fine_feats, coarse_ids, num_coarse):
    global LAST_RESULT
    from concourse.bass_utils import run_bass_kernel_spmd

    cfg = CFG
    # guard: if the data needs more slots or wider groups than compiled,
    # escalate the config (never triggers for uniform-random ids)
    ids64 = np.asarray(coarse_ids, dtype=np.int64).ravel()
    assert int(num_coarse) <= cfg.n_coarse_pad and (
        ids64.size == 0 or int(ids64.max()) < cfg.n_coarse_pad
    ), "compiled for num_coarse <= 250880"
    owner = ids64 // cfg.rng
    nmax = int(np.bincount(owner, minlength=NCORES).max())
    s_need = max(cfg.s_slots, -(-nmax // 128))
    s_need += s_need % 2
    if s_need > cfg.s_slots:
        cfg = Cfg(s_slots=s_need)
    in_maps, bases_all, spill = shard_inputs(fine_feats, coarse_ids, cfg)
    key = ("v3", cfg.s_slots, cfg.w)
    if key not in _nc_cache:
        _nc_cache[key] = build_nc(cfg)
    nc = _nc_cache[key]
    trace = bool(int(os.environ.get("KERNEL_TRACE", "0")))
    if trace:
        _install_axon_hooks_shim()
    res = run_bass_kernel_spmd(nc, in_maps, core_ids=list(range(NCORES)), trace=trace)
    LAST_RESULT = res
    return assemble_output(res.results, bases_all, spill, int(num_coarse), cfg)


# revision 7
# speedup vs baseline: 1.0006x; 1.0006x over previous
"""Sparse avg-pool (segment mean) for Trainium2, 8 NeuronCores.

Strategy: range-shard coarse ids across cores (core k owns ids
[k*31360, (k+1)*31360)), so no collective is needed.  Host-side prep per
core: prescale each token's features by 1/count(coarse id) (counts fall out
of routing), sort tokens by id, and pack them densely into slots of 128.
Every K=2 consecutive slots (256 tokens) form a group with a data-dependent
base id; the device builds a [128 token, W=32 seg] one-hot from
group-relative ids per slot and the PE accumulates onehot^T @ feats into a
per-group PSUM tile.  Because the sum is prescaled, PSUM holds the mean
directly: ACT copies it to SBUF as bf16 and it DMAs straight out.  Adjacent
groups may overlap in id range; the host adds group outputs into the final
array (partial sums, exact).  W=32 deliberately undercuts the max group
span (40 for uniform data): the ~3% of tokens past the window boundary are
blanked on the device (id -1 never matches) and added on the host during
assembly, like halo cells — trimming output rows shipped from 40 to 32 per
group.

Performance notes (TRN2): the fused is_equal hits the DVE 2x_1p fast mode
(all operands 2-byte, unit last-axis stride) via the [p, group, seg, slot]
one-hot layout and a [128, W, K] materialized iota; LDWEIGHTS+MATMUL pairs
issue every ~40-55 ns when the lhsT free-axis stride is small; input streams
chunk-major from DRAM on the sync HWDGE ring while outputs ride the gpsimd
SWDGE ring (last chunks on sync to cut the tail).  ~126 us vs the 364 us
one-hot-on-DVE baseline; the wall is the 36.7 MB blended bf16 stream at the
~332 GB/s practical DMA ceiling plus ~9 us NEFF startup.
"""
import os
import sys
from dataclasses import dataclass

sys.path.insert(0, "/opt/trn_rl_repo")

import numpy as np

NCORES = 8
C = 64
K = 2          # slots per group
SPG = 128 * K  # tokens per group


@dataclass(frozen=True)
class Cfg:
    n_coarse_pad: int = 250_880
    s_slots: int = 1964   # slots per core (982 groups of 2)
    w: int = 32           # one-hot window width (boundary tokens spill to host)
    spc0: int = 32        # starter chunk size (slots)
    spc: int = 64         # main chunk size (slots)
    drain: int = 16       # groups per PSUM tile / ACT copy
    dma_grp: int = 32     # groups per output DMA

    @property
    def rng(self):
        return self.n_coarse_pad // NCORES

    @property
    def n_grp(self):
        return self.s_slots // K

    @property
    def s_tot(self):
        return self.s_slots * 128


CFG = Cfg()
_nc_cache = {}
LAST_RESULT = None


def chunk_plan(cfg: Cfg):
    """Chunk sizes in slots: two small starter chunks to fill the pipeline
    fast, then large transfers (+ ragged even tail) to amortize DMA setup."""
    rem = cfg.s_slots - 2 * cfg.spc0
    assert rem > 0 and rem % 2 == 0
    plan = [cfg.spc0, cfg.spc0] + [cfg.spc] * (rem // cfg.spc)
    tail = rem % cfg.spc
    if tail:
        plan.append(tail)
    return plan


def build_nc(cfg: Cfg):
    from concourse import bacc, mybir, tile

    bf16 = mybir.dt.bfloat16
    f32 = mybir.dt.float32
    nc = bacc.Bacc("TRN2", target_bir_lowering=False)
    plan = chunk_plan(cfg)
    mspc0 = max(plan)
    feats_ext = nc.declare_dram_parameter(
        "feats", [len(plan), 128, mspc0 * C], bf16, isOutput=False
    )
    ids_ext = nc.declare_dram_parameter(
        "ids", [128, cfg.s_slots], bf16, isOutput=False
    )
    iota_ext = nc.declare_dram_parameter(
        "iota", [128, cfg.w, K], bf16, isOutput=False
    )
    out_ext = nc.declare_dram_parameter(
        "out", [cfg.w, cfg.n_grp, C], bf16, isOutput=True
    )

    mspc = max(plan)
    mcg = mspc // K

    with tile.TileContext(nc) as tc:
        with (
            tc.tile_pool(name="stage", bufs=8) as stagep,
            tc.tile_pool(name="oh", bufs=6) as ohp,
            tc.tile_pool(name="psum", bufs=4, space="PSUM") as psump,
            tc.tile_pool(name="fin", bufs=6) as finp,
            tc.tile_pool(name="cst", bufs=1) as cstp,
        ):
            iota_t = cstp.tile([128, cfg.w, K], bf16)
            nc.sync.dma_start(out=iota_t[:], in_=iota_ext[:])
            ids_t = cstp.tile([128, cfg.s_slots], bf16)
            nc.sync.dma_start(out=ids_t[:], in_=ids_ext[:])

            col = 0
            g = 0
            deferred = []
            for ci, spc in enumerate(plan):
                cg = spc // K
                s0 = col
                blk = stagep.tile([128, mspc, C], bf16, tag="src")
                nc.sync.dma_start(
                    out=blk[:, :spc, :],
                    in_=feats_ext[ci, :, : spc * C].rearrange(
                        "p (t c) -> p t c", c=C
                    ),
                )
                feats_v = blk[:, :spc, :]
                ids_v = ids_t[:, s0 : s0 + spc]
                oh = ohp.tile([128, mcg, cfg.w, K], bf16, tag="oh")
                nc.vector.tensor_tensor(
                    out=oh[:, :cg],
                    in0=ids_v.rearrange("p (g s) -> p g s", g=cg)
                    .unsqueeze(2)
                    .to_broadcast([128, cg, cfg.w, K]),
                    in1=iota_t[:]
                    .unsqueeze(1)
                    .to_broadcast([128, cg, cfg.w, K]),
                    op=mybir.AluOpType.is_equal,
                )
                dma_grp = cfg.drain if ci == len(plan) - 1 else cfg.dma_grp
                for h0 in range(0, cg, dma_grp):
                    hn = min(dma_grp, cg - h0)
                    ot = finp.tile([cfg.w, cfg.dma_grp, C], bf16, tag="ot")
                    for d0 in range(h0, h0 + hn, cfg.drain):
                        dn = min(cfg.drain, h0 + hn - d0)
                        ps = psump.tile([cfg.w, cfg.drain, C], f32, tag="ps")
                        for gi in range(dn):
                            for s in range(K):
                                t = (d0 + gi) * K + s
                                nc.tensor.matmul(
                                    out=ps[:, gi, :],
                                    lhsT=oh[:, d0 + gi, :, s],
                                    rhs=feats_v[:, t, :],
                                    start=(s == 0),
                                    stop=(s == K - 1),
                                )
                        nc.scalar.activation(
                            ot[:, d0 - h0 : d0 - h0 + dn, :],
                            ps[:, :dn, :],
                            mybir.ActivationFunctionType.Copy,
                        )
                    if ci >= len(plan) - 3:
                        # defer: the sync ring is in-order, so a compute-
                        # dependent output here would block the remaining
                        # input chunks queued behind it
                        deferred.append((g + h0, hn, ot))
                    else:
                        nc.gpsimd.dma_start(
                            out=out_ext[:, g + h0 : g + h0 + hn, :],
                            in_=ot[:, :hn, :],
                        )
                col += spc
                g += cg
            for g0, hn, ot in deferred:
                nc.sync.dma_start(
                    out=out_ext[:, g0 : g0 + hn, :],
                    in_=ot[:, :hn, :],
                )
    nc.compile()
    return nc


def shard_inputs(feats, ids, cfg: Cfg):
    """Host: scale by 1/count, route to owner cores, sort by local id,
    pack densely, compute per-group base ids and group-relative ids."""
    import ml_dtypes

    ids = np.asarray(ids, dtype=np.int64).ravel()
    feats = np.asarray(feats, dtype=np.float32)
    cnt = np.bincount(ids, minlength=cfg.n_coarse_pad).astype(np.float32)
    scale = 1.0 / np.maximum(cnt, 1.0)
    feats = feats * scale[ids][:, None]

    owner = ids // cfg.rng
    local = (ids - owner * cfg.rng).astype(np.int64)
    order = np.argsort(owner, kind="stable")
    counts = np.bincount(owner, minlength=NCORES)
    offs = np.zeros(NCORES + 1, np.int64)
    np.cumsum(counts, out=offs[1:])
    feats_sorted = feats[order]
    local_sorted = local[order]

    iota = np.broadcast_to(
        np.arange(cfg.w, dtype=np.float32)[None, :, None], (128, cfg.w, K)
    ).astype(ml_dtypes.bfloat16)
    iota = np.ascontiguousarray(iota)

    in_maps = []
    bases_all = []
    spill_idx = []
    spill_val = []
    for k in range(NCORES):
        fk = feats_sorted[offs[k] : offs[k + 1]]
        lk = local_sorted[offs[k] : offs[k + 1]]
        n_k = lk.shape[0]
        assert n_k <= cfg.s_tot, f"core {k}: {n_k} tokens > {cfg.s_tot}"
        sorder = np.argsort(lk, kind="stable")
        ls = lk[sorder]
        fs = fk[sorder]
        bases = np.zeros(cfg.n_grp, np.int64)
        ngrp_used = -(-n_k // SPG)
        bases[:ngrp_used] = ls[np.arange(ngrp_used) * SPG]
        rel = ls - bases[np.arange(n_k) // SPG]
        # tokens whose group-relative id falls outside the one-hot window are
        # rare (never, for uniform-random ids); blank them in place (the
        # device ignores id -1) and accumulate them on the host instead
        over = rel >= cfg.w
        if over.any():
            spill_idx.append(k * cfg.rng + ls[over])
            spill_val.append(fs[over].copy())
            fs = fs.copy()
            rel = rel.copy()
            fs[over] = 0.0
            rel[over] = -1
        fa = np.zeros((cfg.s_tot, C), np.float32)
        ia = np.full((cfg.s_tot,), -1.0, np.float32)
        fa[:n_k] = fs
        ia[:n_k] = rel.astype(np.float32)
        fa = fa.reshape(cfg.s_slots, 128, C).transpose(1, 0, 2)  # [128, S, C]
        plan = chunk_plan(cfg)
        mspc = max(plan)
        feats_arr = np.zeros((len(plan), 128, mspc * C), ml_dtypes.bfloat16)
        s0 = 0
        for ci, spc in enumerate(plan):
            feats_arr[ci, :, : spc * C] = fa[:, s0 : s0 + spc, :].reshape(
                128, spc * C
            )
            s0 += spc
        ids_arr = np.ascontiguousarray(
            ia.reshape(cfg.s_slots, 128).T
        ).astype(ml_dtypes.bfloat16)
        in_maps.append({"feats": feats_arr, "ids": ids_arr, "iota": iota})
        bases_all.append(bases)
    if spill_idx:
        spill = (np.concatenate(spill_idx), np.concatenate(spill_val))
    else:
        spill = None
    return in_maps, bases_all, spill


def assemble_output(results, bases_all, spill, n_coarse, cfg: Cfg):
    out = np.empty((NCORES * cfg.rng, C), np.float32)
    for k in range(NCORES):
        dev = np.asarray(results[k]["out"], dtype=np.float32)  # [W, n_grp, C]
        acc = np.zeros((cfg.rng + cfg.w, C), np.float32)
        bases = bases_all[k]
        for g in range(cfg.n_grp):
            b = bases[g]
            acc[b : b + cfg.w] += dev[:, g, :]
        out[k * cfg.rng : (k + 1) * cfg.rng] = acc[: cfg.rng]
    if spill is not None:
        np.add.at(out, spill[0], spill[1])
    return out[:n_coarse]


def _install_axon_hooks_shim():
    """Provide antenv.axon_hooks + the ctypes NTFF hook if the image lacks it."""
    import contextlib
    import ctypes
    import types

    try:
        from antenv.axon_hooks import get_axon_ntff_profile_hook  # noqa: F401

        return
    except ImportError:
        pass
    import antenv

    mod = types.ModuleType("antenv.axon_hooks")
    state = {"h": None}
    mod.set_axon_ntff_profile_hook = lambda h: state.__setitem__("h", h)
    mod.get_axon_ntff_profile_hook = lambda: state["h"]
    antenv.axon_hooks = mod
    sys.modules["antenv.axon_hooks"] = mod

    so_path = "/opt/axon/libaxon_pjrt.so"
    if not os.path.exists(so_path):
        return
    lib = ctypes.CDLL(so_path)
    if not hasattr(lib, "axon_start_nrt_profile"):
        return
    lib.axon_start_nrt_profile.argtypes = [
        ctypes.POINTER(ctypes.c_int64),
        ctypes.c_size_t,
    ]
    lib.axon_start_nrt_profile.restype = ctypes.c_int64
    lib.axon_stop_nrt_profile.argtypes = [ctypes.c_char_p]
    lib.axon_stop_nrt_profile.restype = ctypes.c_int64

    @contextlib.contextmanager
    def _hook(output_dir, device_ids):
        import jax

        jax.devices()
        if device_ids:
            ids = (ctypes.c_int64 * len(device_ids))(*device_ids)
            rc = lib.axon_start_nrt_profile(ids, len(device_ids))
        else:
            rc = lib.axon_start_nrt_profile(None, 0)
        if rc != 0:
            raise RuntimeError(f"axon_start_nrt_profile rc={rc}")
        try:
            yield
        finally:
            n = lib.axon_stop_nrt_profile(str(output_dir).encode())
            print(f"profile: {n} file(s) written to {output_dir}", file=sys.stderr)

    state["h"] = _hook


def kernel(fine_feats, coarse_ids, num_coarse):
    global LAST_RESULT
    from concourse.bass_utils import run_bass_kernel_spmd

    cfg = CFG
    # guard: if the data needs more slots or wider groups than compiled,
    # escalate the config (never triggers for uniform-random ids)
    ids64 = np.asarray(coarse_ids, dtype=np.int64).ravel()
    assert int(num_coarse) <= cfg.n_coarse_pad and (
        ids64.size == 0 or int(ids64.max()) < cfg.n_coarse_pad
    ), "compiled for num_coarse <= 250880"
    owner = ids64 // cfg.rng
    nmax = int(np.bincount(owner, minlength=NCORES).max())
    s_need = max(cfg.s_slots, -(-nmax // 128))
    s_need += s_need % 2
    if s_need > cfg.s_slots:
        cfg = Cfg(s_slots=s_need)
    in_maps, bases_all, spill = shard_inputs(fine_feats, coarse_ids, cfg)
    key = ("v3", cfg.s_slots, cfg.w)
    if key not in _nc_cache:
        _nc_cache[key] = build_nc(cfg)
    nc = _nc_cache[key]
    trace = bool(int(os.environ.get("KERNEL_TRACE", "0")))
    if trace:
        _install_axon_hooks_shim()
    res = run_bass_kernel_spmd(nc, in_maps, core_ids=list(range(NCORES)), trace=trace)
    LAST_RESULT = res
    return assemble_output(res.results, bases_all, spill, int(num_coarse), cfg)
